# revision 20
# baseline (speedup 1.0000x reference)
"""MoE gate routing kernel for Trainium2 (Bass/Tile), 8-way token-sharded.

Computes, for x = hidden_states.reshape(-1, H) and gate weight W [E, H]:
    logits = x @ W.T            # [T, E]
    top-8 of softmax(logits) with renormalized weights
Returns (topk_weight [T, 8] f32, topk_idx [T, 8] i32), matching the reference.

Math note: softmax then top-k + renormalize equals top-k on logits followed
by softmax over just those 8 logits (the global partition function cancels;
the reference's +1e-20 is negligible since the max prob >= 1/64).

Precision: x and W are split on the host into bf16 hi + bf16 lo halves
(x ~= xh + xl to ~2^-18 relative). logits are computed as three accumulating
bf16 matmul chains xh@wh + xh@wl + xl@wh in fp32 PSUM; the dropped xl@wl
term is ~2^-18 relative, giving near-fp32 logits (top-8 flips only on
near-exact ties; simulated rel_i ~4e-3, well under the 2e-2 gate). bf16
matmuls stream 1 column/cycle on the PE where plain fp32 needs 4.

Layout: the host transposes x to xT [H, T] and ships bf16 halves, so the
kernel needs NO on-device transposes of x (the dominant PE cost of an
fp32 version). DMA traffic is unchanged vs fp32 x (2 halves x 2 bytes).

Per-core schedule (2048 tokens = 4 slabs x 512 tokens):
  - wTh/wTl staged in SBUF once ([128, 32*64] chunk-major).
  - Per slab: 8 DMAs (4 sub-blocks x {xh, xl}), each [128, 8x512] bf16;
    96 accumulating matmuls (3 chains x 32 k-chunks) into PSUM [64, 512];
    matmuls chase the sub-block DMAs so the PE rarely idles.
  - Epilogue per slab: DVE copy of logits^T to SBUF, 4 exact fp32 PE
    transposes back to [tokens, experts], ACT copy, then per 128-token
    quarter: DVE max8 + max-index, ACT exp with accumulate, DVE
    reciprocal + scale, DMA out.
"""

import numpy as np

import concourse.bass as bass
import concourse.mybir as mybir
from concourse import masks
from concourse.bass_utils import run_bass_kernel_spmd
from concourse.tile import TileContext

P = 128          # SBUF partitions
H = 4096         # hidden dim
E = 64           # experts
K = 8            # top-k
N_CORES = 8
T_TOTAL = 4 * 4096
T_CORE = T_TOTAL // N_CORES   # 2048
SLAB = 4 * P                  # 512 tokens per slab
N_SLAB = T_CORE // SLAB       # 4
N_KC = H // P                 # 32 contraction chunks
N_SUB = 4                     # DMA sub-blocks per slab
KC_SUB = N_KC // N_SUB        # 8 chunks per sub-block

F32 = mybir.dt.float32
BF16 = mybir.dt.bfloat16
U32 = mybir.dt.uint32
EXP = mybir.ActivationFunctionType.Exp


def build_bass(loop_reps=None, xin_bufs=2 * N_SUB, lgt_bufs=2):
    nc = bass.Bass()
    # x^T bf16 halves, chunk-major: [kc, p, t] with h = kc*128 + p
    xh = nc.declare_dram_parameter("xh", [N_KC, P, T_CORE], BF16, isOutput=False)
    xl = nc.declare_dram_parameter("xl", [N_KC, P, T_CORE], BF16, isOutput=False)
    # W^T bf16 halves, same chunk-major layout: [kc, p, e]
    wh = nc.declare_dram_parameter("wh", [N_KC, P, E], BF16, isOutput=False)
    wl = nc.declare_dram_parameter("wl", [N_KC, P, E], BF16, isOutput=False)
    out_w = nc.declare_dram_parameter("out_w", [T_CORE, K], F32, isOutput=True)
    out_i = nc.declare_dram_parameter("out_i", [T_CORE, K], U32, isOutput=True)

    with TileContext(nc) as tc:
        with (
            tc.tile_pool(name="singles", bufs=1) as singles,
            tc.tile_pool(name="xin", bufs=xin_bufs) as x_pool,
            tc.tile_pool(name="lgtp", bufs=lgt_bufs, space="PSUM") as lgt_psum,
            tc.tile_pool(name="mgp", bufs=1, space="PSUM") as mg_psum,
            tc.tile_pool(name="lgqp", bufs=2, space="PSUM") as lgq_psum,
            tc.tile_pool(name="sm", bufs=3) as sm_pool,
        ):
            identity = singles.tile([P, P], F32)
            masks.make_identity(nc, identity[:])
            # merge operand: [I64; I64] stacked -> adds PSUM halves
            mergeM = singles.tile([P, E], F32)
            nc.gpsimd.memset(mergeM[:], 0.0)
            masks.make_identity(nc, mergeM[0:E, 0:E], nomemset=True)
            masks.make_identity(nc, mergeM[E : 2 * E, 0:E], nomemset=True)

            whs = singles.tile([P, N_KC * E], BF16)
            wls = singles.tile([P, N_KC * E], BF16)
            nc.sync.dma_start(
                out=whs[:].rearrange("p (kc e) -> p kc e", kc=N_KC),
                in_=wh[:, :, :].transpose([1, 0, 2]),
            )
            nc.sync.dma_start(
                out=wls[:].rearrange("p (kc e) -> p kc e", kc=N_KC),
                in_=wl[:, :, :].transpose([1, 0, 2]),
            )

            def do_slab(s):
                t0 = s * SLAB
                xh_t, xl_t = [], []
                for b in range(N_SUB):
                    xht = x_pool.tile([P, KC_SUB * SLAB], BF16, tag="xh")
                    nc.sync.dma_start(
                        out=xht[:].rearrange("p (kc t) -> p kc t", kc=KC_SUB),
                        in_=xh[
                            b * KC_SUB : (b + 1) * KC_SUB, :, t0 : t0 + SLAB
                        ].transpose([1, 0, 2]),
                    )
                    xh_t.append(xht)
                    xlt = x_pool.tile([P, KC_SUB * SLAB], BF16, tag="xl")
                    nc.sync.dma_start(
                        out=xlt[:].rearrange("p (kc t) -> p kc t", kc=KC_SUB),
                        in_=xl[
                            b * KC_SUB : (b + 1) * KC_SUB, :, t0 : t0 + SLAB
                        ].transpose([1, 0, 2]),
                    )
                    xl_t.append(xlt)

                # Three bf16 chains (xh@wh + xl@wh + xh@wl) col-paired on the
                # 128-wide PE array: M=64 uses only half the columns, so two
                # matmuls run concurrently via tile_position (0,0)/(0,64).
                # Alternate which half carries 2-of-3 per k-chunk so both
                # halves do 1.5 matmuls/chunk; partials split arbitrarily
                # between PSUM halves and are summed in the transpose stage.
                lgt = lgt_psum.tile([P, SLAB], F32)
                first = {0: True, 1: True}
                seq = []
                for kc in range(N_KC):
                    b, j = divmod(kc, KC_SUB)
                    xh_mov = xh_t[b][:, j * SLAB : (j + 1) * SLAB]
                    xl_mov = xl_t[b][:, j * SLAB : (j + 1) * SLAB]
                    w_hi = whs[:, kc * E : (kc + 1) * E]
                    w_lo = wls[:, kc * E : (kc + 1) * E]
                    if kc % 2 == 0:
                        seq += [(0, w_hi, xh_mov), (1, w_lo, xh_mov),
                                (0, w_hi, xl_mov)]
                    else:
                        seq += [(1, w_hi, xh_mov), (0, w_lo, xh_mov),
                                (1, w_hi, xl_mov)]
                last_of = {0: max(i for i, m in enumerate(seq) if m[0] == 0),
                           1: max(i for i, m in enumerate(seq) if m[0] == 1)}
                for i, (half, w_st, x_mv) in enumerate(seq):
                    # per-half start/stop (each clears/ends its own partition
                    # range's has_written); CoreSim's group check keys zero
                    # regions without base partition, so skip it.
                    nc.tensor.matmul(
                        lgt[half * E : (half + 1) * E, :], w_st, x_mv,
                        start=first[half], stop=(i == last_of[half]),
                        tile_position=(0, half * E),
                        skip_group_check=True,
                    )
                    first[half] = False

                # merge halves exactly (fp32 matmul with stacked identities),
                # then exact fp32 transposes back to [tokens, experts]
                lgt_sb = sm_pool.tile([P, SLAB], F32, tag="lgt_sb")
                nc.vector.tensor_copy(lgt_sb[:], lgt[:])
                mg = mg_psum.tile([E, SLAB], F32, tag="epi")
                nc.tensor.matmul(
                    mg[:], mergeM[:], lgt_sb[:], start=True, stop=True,
                    tile_position=(0, 0),
                )
                mg_sb = sm_pool.tile([E, SLAB], F32, tag="mg_sb")
                nc.vector.tensor_copy(mg_sb[:], mg[:])
                lgq = lgq_psum.tile([P, 4 * E], F32)
                for q in range(4):
                    nc.tensor.matmul(
                        lgq[:, q * E : (q + 1) * E],
                        mg_sb[:, q * P : (q + 1) * P],
                        identity[:E, :E],
                        is_transpose=True,
                        start=(q == 0),
                        stop=(q == 3),
                    )
                lg_sb = sm_pool.tile([P, 4 * E], F32, tag="lg_sb")
                nc.scalar.copy(lg_sb[:], lgq[:])

                for q in range(4):
                    lg = lg_sb[:, q * E : (q + 1) * E]
                    t8v = sm_pool.tile([P, K], F32, tag="t8v")
                    nc.vector.max(out=t8v[:], in_=lg)
                    t8i = sm_pool.tile([P, K], U32, tag="t8i")
                    nc.vector.max_index(out=t8i[:], in_max=t8v[:], in_values=lg)
                    nmax = sm_pool.tile([P, 1], F32, tag="nmax")
                    nc.vector.tensor_scalar_mul(nmax[:], t8v[:, 0:1], -1.0)
                    e8 = sm_pool.tile([P, K], F32, tag="e8")
                    s1 = sm_pool.tile([P, 1], F32, tag="s1")
                    nc.scalar.activation(
                        e8[:], t8v[:], EXP, bias=nmax[:], scale=1.0, accum_out=s1[:]
                    )
                    r1 = sm_pool.tile([P, 1], F32, tag="r1")
                    nc.vector.reciprocal(r1[:], s1[:])
                    w8 = sm_pool.tile([P, K], F32, tag="w8")
                    nc.vector.tensor_scalar_mul(w8[:], e8[:], r1[:])
                    tq = s * SLAB + q * P
                    nc.sync.dma_start(out=out_w[tq : tq + P, :], in_=w8[:])
                    nc.sync.dma_start(out=out_i[tq : tq + P, :], in_=t8i[:])

            def main_body():
                for s in range(N_SLAB):
                    do_slab(s)

            if loop_reps is None:
                main_body()
            else:
                with tc.For_i(0, loop_reps, 1):
                    main_body()

    _legalize_waits(nc)
    return nc


def _legalize_waits(nc):
    """Walrus allows only one sem wait on most instruction structs (matmul
    weight-load, DVE/ACT compute, pseudo-DMA, drain). Tile sometimes emits
    more. Fix: hoist excess waits onto standalone EventSemaphore instructions
    inserted just before the owner in its engine stream (same engine ->
    in-order issue preserves semantics)."""
    n = 0
    for f in nc.m.functions:
        for blk in f.blocks:
            out = []
            changed = False
            for i in blk.instructions:
                si = getattr(i, "sync_info", None)
                ow = list(si.on_wait) if (si is not None and si.on_wait) else []
                if len(ow) > 1:
                    while len(ow) > 1:
                        w = ow.pop(0)
                        out.append(
                            mybir.InstEventSemaphore(
                                name=f"I-whoist-{n}",
                                engine=i.engine,
                                ins=[],
                                outs=[],
                                sync_info=mybir.SyncInfo(on_wait=[w], on_update=[]),
                            )
                        )
                        n += 1
                    si.on_wait = ow
                    changed = True
                out.append(i)
            if changed:
                blk.instructions = out
    return nc


def _bf16_split(a_f32):
    """Split fp32 array into (hi, lo) bf16 halves, RNE, as bf16 views."""
    import ml_dtypes

    def rne_bf16(f):
        bits = f.view(np.uint32)
        lsb = (bits >> np.uint32(16)) & np.uint32(1)
        rnd = bits + np.uint32(0x7FFF) + lsb
        return (rnd >> np.uint32(16)).astype(np.uint16)

    hi_u16 = rne_bf16(a_f32)
    hi_f32 = (hi_u16.astype(np.uint32) << np.uint32(16)).view(np.float32)
    lo_u16 = rne_bf16(a_f32 - hi_f32)
    return hi_u16.view(ml_dtypes.bfloat16), lo_u16.view(ml_dtypes.bfloat16)


_NC = None


def _get_nc():
    global _NC
    if _NC is None:
        _NC = build_bass()
    return _NC


def host_prepare(hidden_states, weight):
    """Shard + transpose + bf16-split the full inputs into per-core maps."""
    hs = np.asarray(hidden_states, dtype=np.float32).reshape(T_TOTAL, H)
    wt = np.ascontiguousarray(np.asarray(weight, dtype=np.float32).T)  # [H, E]
    wh_u, wl_u = _bf16_split(wt)
    wh3 = np.ascontiguousarray(wh_u.reshape(N_KC, P, E))
    wl3 = np.ascontiguousarray(wl_u.reshape(N_KC, P, E))

    in_maps = []
    for c in range(N_CORES):
        xc = np.ascontiguousarray(hs[c * T_CORE : (c + 1) * T_CORE, :].T)  # [H, Tc]
        xh_u, xl_u = _bf16_split(xc)
        in_maps.append(
            {
                "xh": xh_u.reshape(N_KC, P, T_CORE),
                "xl": xl_u.reshape(N_KC, P, T_CORE),
                "wh": wh3,
                "wl": wl3,
            }
        )
    return in_maps


def kernel(hidden_states, weight, **run_kwargs):
    in_maps = host_prepare(hidden_states, weight)
    nc = _get_nc()
    res = run_bass_kernel_spmd(nc, in_maps, core_ids=list(range(N_CORES)), **run_kwargs)
    topk_weight = np.concatenate([r["out_w"] for r in res.results], axis=0)
    topk_idx = np.concatenate(
        [r["out_i"].astype(np.int32) for r in res.results], axis=0
    )
    if run_kwargs:
        kernel.last_result = res
    return topk_weight, topk_idx


# revision 27
# speedup vs baseline: 1.3210x; 1.3210x over previous
"""MoE gate routing kernel for Trainium2 (Bass/Tile), 8-way token-sharded.

Computes, for x = hidden_states.reshape(-1, H) and gate weight W [E, H]:
    logits = x @ W.T            # [T, E]
    top-8 of softmax(logits) with renormalized weights
Returns (topk_weight [T, 8] f32, topk_idx [T, 8] i32), matching the reference.

Math note: softmax then top-k + renormalize equals top-k on logits followed
by softmax over just those 8 logits (the global partition function cancels;
the reference's +1e-20 is negligible since the max prob >= 1/64).

Precision: x and W are split on the host into bf16 hi + bf16 lo halves
(x ~= xh + xl to ~2^-18 relative). logits are computed as three accumulating
bf16 matmul chains xh@wh + xh@wl + xl@wh in fp32 PSUM; the dropped xl@wl
term is ~2^-18 relative, giving near-fp32 logits (top-8 flips only on
near-exact ties; simulated rel_i ~4e-3, well under the 2e-2 gate). bf16
matmuls stream 1 column/cycle on the PE where plain fp32 needs 4.

Layout: the host transposes x to xT [H, T] and ships bf16 halves, so the
kernel needs NO on-device transposes of x (the dominant PE cost of an
fp32 version). DMA traffic is unchanged vs fp32 x (2 halves x 2 bytes).

Per-core schedule (2048 tokens = 4 slabs x 512 tokens):
  - wTh/wTl staged in SBUF once ([128, 32*64] chunk-major).
  - Per slab: 8 DMAs (4 sub-blocks x {xh, xl}), each [128, 8x512] bf16;
    96 accumulating matmuls (3 chains x 32 k-chunks) into PSUM [64, 512];
    matmuls chase the sub-block DMAs so the PE rarely idles.
  - Epilogue per slab: DVE copy of logits^T to SBUF, 4 exact fp32 PE
    transposes back to [tokens, experts], ACT copy, then per 128-token
    quarter: DVE max8 + max-index, ACT exp with accumulate, DVE
    reciprocal + scale, DMA out.
"""

import numpy as np

import concourse.bass as bass
import concourse.mybir as mybir
from concourse import masks
from concourse.bass_utils import run_bass_kernel_spmd
from concourse.tile import TileContext

P = 128          # SBUF partitions
H = 4096         # hidden dim
E = 64           # experts
K = 8            # top-k
N_CORES = 8
T_TOTAL = 4 * 4096
T_CORE = T_TOTAL // N_CORES   # 2048
SLAB = 4 * P                  # 512 tokens per slab
N_SLAB = T_CORE // SLAB       # 4
N_KC = H // P                 # 32 contraction chunks
N_SUB = 4                     # DMA sub-blocks per slab
KC_SUB = N_KC // N_SUB        # 8 chunks per sub-block

F32 = mybir.dt.float32
BF16 = mybir.dt.bfloat16
U32 = mybir.dt.uint32
EXP = mybir.ActivationFunctionType.Exp


def build_bass(loop_reps=None, xin_bufs=None, lgt_bufs=4, pair=True, n_sub=N_SUB,
               xl_on_act=False, sm_bufs=3):
    kc_sub = N_KC // n_sub
    if xin_bufs is None:
        xin_bufs = 2 * n_sub
    nc = bass.Bass()
    # x^T bf16 halves, slab-major so each DMA line is 8KB contiguous:
    # element [s, p, kc, t] = xT[kc*128 + p, s*SLAB + t]
    xh = nc.declare_dram_parameter("xh", [N_SLAB, P, N_KC, SLAB], BF16, isOutput=False)
    xl = nc.declare_dram_parameter("xl", [N_SLAB, P, N_KC, SLAB], BF16, isOutput=False)
    # W^T bf16 halves, same chunk-major layout: [kc, p, e]
    wh = nc.declare_dram_parameter("wh", [N_KC, P, E], BF16, isOutput=False)
    wl = nc.declare_dram_parameter("wl", [N_KC, P, E], BF16, isOutput=False)
    out_w = nc.declare_dram_parameter("out_w", [T_CORE, K], F32, isOutput=True)
    out_i = nc.declare_dram_parameter("out_i", [T_CORE, K], U32, isOutput=True)

    with TileContext(nc) as tc:
        with (
            tc.tile_pool(name="singles", bufs=1) as singles,
            tc.tile_pool(name="xin", bufs=xin_bufs) as x_pool,
            tc.tile_pool(name="lgtp", bufs=lgt_bufs, space="PSUM") as lgt_psum,
            tc.tile_pool(name="mgp", bufs=1, space="PSUM") as mg_psum,
            tc.tile_pool(name="lgqp", bufs=2, space="PSUM") as lgq_psum,
            tc.tile_pool(name="sm", bufs=sm_bufs) as sm_pool,
        ):
            identity = singles.tile([P, P], F32)
            masks.make_identity(nc, identity[:])
            # merge operand: [I64; I64] stacked -> adds PSUM halves
            mergeM = singles.tile([P, E], F32)
            nc.gpsimd.memset(mergeM[:], 0.0)
            masks.make_identity(nc, mergeM[0:E, 0:E], nomemset=True)
            masks.make_identity(nc, mergeM[E : 2 * E, 0:E], nomemset=True)

            whs = singles.tile([P, N_KC * E], BF16)
            wls = singles.tile([P, N_KC * E], BF16)
            nc.sync.dma_start(
                out=whs[:].rearrange("p (kc e) -> p kc e", kc=N_KC),
                in_=wh[:, :, :].transpose([1, 0, 2]),
            )
            nc.sync.dma_start(
                out=wls[:].rearrange("p (kc e) -> p kc e", kc=N_KC),
                in_=wl[:, :, :].transpose([1, 0, 2]),
            )

            def emit_dma(s):
                xh_t, xl_t = [], []
                for b in range(n_sub):
                    xht = x_pool.tile([P, kc_sub * SLAB], BF16, tag="xh")
                    nc.sync.dma_start(
                        out=xht[:],
                        in_=xh[s, :, b * kc_sub : (b + 1) * kc_sub, :].rearrange(
                            "p kc t -> p (kc t)"
                        ),
                    )
                    xh_t.append(xht)
                    xlt = x_pool.tile([P, kc_sub * SLAB], BF16, tag="xl")
                    xl_eng = nc.scalar if xl_on_act else nc.sync
                    xl_eng.dma_start(
                        out=xlt[:],
                        in_=xl[s, :, b * kc_sub : (b + 1) * kc_sub, :].rearrange(
                            "p kc t -> p (kc t)"
                        ),
                    )
                    xl_t.append(xlt)
                return xh_t, xl_t

            def emit_mains(xh_t, xl_t):
                # Three bf16 chains (xh@wh + xl@wh + xh@wl) col-paired on the
                # 128-wide PE array: M=64 uses only half the columns, so two
                # matmuls run concurrently via tile_position (0,0)/(0,64).
                # Alternate which half carries 2-of-3 per k-chunk so both
                # halves do 1.5 matmuls/chunk; partials split arbitrarily
                # between PSUM halves and are summed in the merge stage.
                first = {0: True, 1: True}
                seq = []
                for kc in range(N_KC):
                    b, j = divmod(kc, kc_sub)
                    xh_mov = xh_t[b][:, j * SLAB : (j + 1) * SLAB]
                    xl_mov = xl_t[b][:, j * SLAB : (j + 1) * SLAB]
                    w_hi = whs[:, kc * E : (kc + 1) * E]
                    w_lo = wls[:, kc * E : (kc + 1) * E]
                    if not pair:
                        seq += [(0, w_hi, xh_mov), (0, w_hi, xl_mov),
                                (0, w_lo, xh_mov)]
                    elif kc % 2 == 0:
                        seq += [(0, w_hi, xh_mov), (1, w_lo, xh_mov),
                                (0, w_hi, xl_mov)]
                    else:
                        seq += [(1, w_hi, xh_mov), (0, w_lo, xh_mov),
                                (1, w_hi, xl_mov)]
                lgt = lgt_psum.tile([P if pair else E, SLAB], F32)
                last_of = {h: max((i for i, m in enumerate(seq) if m[0] == h),
                                  default=None)
                           for h in (0, 1)}
                for i, (half, w_st, x_mv) in enumerate(seq):
                    # per-half start/stop (each clears/ends its own partition
                    # range's has_written); CoreSim's group check keys zero
                    # regions without base partition, so skip it.
                    nc.tensor.matmul(
                        lgt[half * E : (half + 1) * E, :], w_st, x_mv,
                        start=first[half], stop=(i == last_of[half]),
                        tile_position=(0, half * E),
                        skip_group_check=True,
                    )
                    first[half] = False
                return lgt

            def emit_epi(s, lgt):
                if pair:
                    # merge halves exactly (fp32 matmul, stacked identities),
                    # then exact fp32 transposes back to [tokens, experts]
                    lgt_sb = sm_pool.tile([P, SLAB], F32, tag="lgt_sb")
                    nc.vector.tensor_copy(lgt_sb[:], lgt[:])
                    mg = mg_psum.tile([E, SLAB], F32, tag="epi")
                    nc.tensor.matmul(
                        mg[:], mergeM[:], lgt_sb[:], start=True, stop=True,
                        tile_position=(0, 0),
                    )
                    mg_sb = sm_pool.tile([E, SLAB], F32, tag="mg_sb")
                    nc.vector.tensor_copy(mg_sb[:], mg[:])
                else:
                    mg_sb = sm_pool.tile([E, SLAB], F32, tag="mg_sb")
                    nc.vector.tensor_copy(mg_sb[:], lgt[:])
                lgq = lgq_psum.tile([P, 4 * E], F32)
                for q in range(4):
                    nc.tensor.matmul(
                        lgq[:, q * E : (q + 1) * E],
                        mg_sb[:, q * P : (q + 1) * P],
                        identity[:E, :E],
                        is_transpose=True,
                        start=(q == 0),
                        stop=(q == 3),
                    )

                # top-8 in phases across the 4 quarters so the in-order DVE
                # stream never head-of-line blocks on the ACT exp
                t8v, t8i, nmax, e8, s1 = [], [], [], [], []
                for q in range(4):
                    lg = lgq[:, q * E : (q + 1) * E]
                    v = sm_pool.tile([P, K], F32, tag="t8v")
                    nc.vector.max(out=v[:], in_=lg)
                    i_ = sm_pool.tile([P, K], U32, tag="t8i")
                    nc.vector.max_index(out=i_[:], in_max=v[:], in_values=lg)
                    m = sm_pool.tile([P, 1], F32, tag="nmax")
                    nc.vector.tensor_scalar_mul(m[:], v[:, 0:1], -1.0)
                    t8v.append(v); t8i.append(i_); nmax.append(m)
                for q in range(4):
                    e = sm_pool.tile([P, K], F32, tag="e8")
                    a = sm_pool.tile([P, 1], F32, tag="s1")
                    nc.scalar.activation(
                        e[:], t8v[q][:], EXP, bias=nmax[q][:], scale=1.0,
                        accum_out=a[:],
                    )
                    e8.append(e); s1.append(a)
                for q in range(4):
                    r1 = sm_pool.tile([P, 1], F32, tag="r1")
                    nc.vector.reciprocal(r1[:], s1[q][:])
                    w8 = sm_pool.tile([P, K], F32, tag="w8")
                    nc.vector.tensor_scalar_mul(w8[:], e8[q][:], r1[:])
                    tq = s * SLAB + q * P
                    # outputs go out on the ACT hwdge so they never block the
                    # SP stream's input prefetch
                    nc.scalar.dma_start(out=out_w[tq : tq + P, :], in_=w8[:])
                    nc.scalar.dma_start(out=out_i[tq : tq + P, :], in_=t8i[q][:])

            def main_body():
                # software pipeline: DMA(s+1) and mains(s) are emitted before
                # epilogue(s-1) so the PE stream never stalls on the epilogue's
                # cross-engine dependency chain
                tiles = {0: emit_dma(0)}
                lgts = {}
                for s in range(N_SLAB):
                    if s + 1 < N_SLAB:
                        tiles[s + 1] = emit_dma(s + 1)
                    lgts[s] = emit_mains(*tiles.pop(s))
                    if s >= 1:
                        emit_epi(s - 1, lgts.pop(s - 1))
                emit_epi(N_SLAB - 1, lgts.pop(N_SLAB - 1))

            if loop_reps is None:
                main_body()
            else:
                with tc.For_i(0, loop_reps, 1):
                    main_body()

    _legalize_waits(nc)
    return nc


def _legalize_waits(nc):
    """Walrus allows only one sem wait on most instruction structs (matmul
    weight-load, DVE/ACT compute, pseudo-DMA, drain). Tile sometimes emits
    more. Fix: hoist excess waits onto standalone EventSemaphore instructions
    inserted just before the owner in its engine stream (same engine ->
    in-order issue preserves semantics)."""
    n = 0
    for f in nc.m.functions:
        for blk in f.blocks:
            out = []
            changed = False
            for i in blk.instructions:
                si = getattr(i, "sync_info", None)
                ow = list(si.on_wait) if (si is not None and si.on_wait) else []
                if len(ow) > 1:
                    while len(ow) > 1:
                        w = ow.pop(0)
                        out.append(
                            mybir.InstEventSemaphore(
                                name=f"I-whoist-{n}",
                                engine=i.engine,
                                ins=[],
                                outs=[],
                                sync_info=mybir.SyncInfo(on_wait=[w], on_update=[]),
                            )
                        )
                        n += 1
                    si.on_wait = ow
                    changed = True
                out.append(i)
            if changed:
                blk.instructions = out
    return nc


def _bf16_split(a_f32):
    """Split fp32 array into (hi, lo) bf16 halves, RNE, as bf16 views."""
    import ml_dtypes

    def rne_bf16(f):
        bits = f.view(np.uint32)
        lsb = (bits >> np.uint32(16)) & np.uint32(1)
        rnd = bits + np.uint32(0x7FFF) + lsb
        return (rnd >> np.uint32(16)).astype(np.uint16)

    hi_u16 = rne_bf16(a_f32)
    hi_f32 = (hi_u16.astype(np.uint32) << np.uint32(16)).view(np.float32)
    lo_u16 = rne_bf16(a_f32 - hi_f32)
    return hi_u16.view(ml_dtypes.bfloat16), lo_u16.view(ml_dtypes.bfloat16)


_NC = None


def _get_nc():
    global _NC
    if _NC is None:
        _NC = build_bass()
    return _NC


def host_prepare(hidden_states, weight):
    """Shard + transpose + bf16-split the full inputs into per-core maps."""
    hs = np.asarray(hidden_states, dtype=np.float32).reshape(T_TOTAL, H)
    wt = np.ascontiguousarray(np.asarray(weight, dtype=np.float32).T)  # [H, E]
    wh_u, wl_u = _bf16_split(wt)
    wh3 = np.ascontiguousarray(wh_u.reshape(N_KC, P, E))
    wl3 = np.ascontiguousarray(wl_u.reshape(N_KC, P, E))

    in_maps = []
    for c in range(N_CORES):
        xc = np.ascontiguousarray(hs[c * T_CORE : (c + 1) * T_CORE, :].T)  # [H, Tc]
        xh_u, xl_u = _bf16_split(xc)

        def slab_major(a):
            a4 = a.reshape(N_KC, P, N_SLAB, SLAB).transpose(2, 1, 0, 3)
            return np.ascontiguousarray(a4)

        in_maps.append(
            {
                "xh": slab_major(xh_u),
                "xl": slab_major(xl_u),
                "wh": wh3,
                "wl": wl3,
            }
        )
    return in_maps


def kernel(hidden_states, weight, **run_kwargs):
    in_maps = host_prepare(hidden_states, weight)
    nc = _get_nc()
    res = run_bass_kernel_spmd(nc, in_maps, core_ids=list(range(N_CORES)), **run_kwargs)
    topk_weight = np.concatenate([r["out_w"] for r in res.results], axis=0)
    topk_idx = np.concatenate(
        [r["out_i"].astype(np.int32) for r in res.results], axis=0
    )
    if run_kwargs:
        kernel.last_result = res
    return topk_weight, topk_idx
